# revision 9
# baseline (speedup 1.0000x reference)
"""Causal multi-head attention (B=2, T=2048, C=1024, H=16, D=64) on 8 TRN2 cores.

Sharding: 2 heads per core (head-parallel). Each core computes its heads'
QKV projection, block-causal attention, and a partial out-projection; the
host sums the 8 partials.

v2 (vs the f32r baseline):
  * bf16 operands everywhere (PSUM stays fp32): halves DMA/SBUF traffic,
    removes the f32r <256-col matmul penalty, keeps 1 cycle/row rate.
  * flat PSUM pools (6+2 banks) + double-buffered cross-batch SBUF tiles so
    the scheduler can interleave QKV(b1) matmuls under the ACT-bound
    attention(b0) inner loop.
  * causal column restriction on the 2nd diagonal pair (scores + exp).
  * triangular mask folded into PSUM via identity-stationary matmuls
    (removes the DVE mask multiply and one act->AV dependency hop).
  * V transposes packed into the spare PSUM bank of the V-chunk tile; one
    batched copy into V_ext per x-chunk.
  * host-side input layouts are partition-contiguous (few, fat DMA
    descriptors) and the first weight/x slivers are split out so the PE
    starts early.
"""
import sys

sys.path.insert(0, "/opt/trn_rl_repo")

import numpy as np
import ml_dtypes

import concourse.bass as bass
import concourse.mybir as mybir
from concourse import bacc
from concourse.tile import TileContext
from concourse.bass_utils import run_bass_kernel_spmd

N_CORES = 8
B, T, C = 2, 2048, 1024
D = 64          # head dim
NH = 2          # heads per core
HC = NH * D     # 128: head-channels per core
BT = B * T      # 4096
TQ = 512        # query tile
NM = T // TQ    # 4 query tiles per batch
NKB = T // 128  # 16 key blocks per batch
NC = B * NM     # 8 x-chunks of 512 tokens
F32 = mybir.dt.float32
BF16 = mybir.dt.bfloat16
SCALE = 1.0 / np.sqrt(D)  # 0.125
BF = ml_dtypes.bfloat16


def build_program(dbg: bool = False):
    nc = bacc.Bacc("TRN2", target_bir_lowering=False, debug=False)

    # xP[p, chunk, o, t] = x.T[o*128+p, chunk*512+t] -- per-partition contiguous
    xP = nc.dram_tensor("xP", [128, NC, 8, TQ], BF16, kind="ExternalInput")
    # wkP[p, o, m] = wk[o*128+p, m], m = [Q(128) | K(128) | V(128)] head cols
    wkP = nc.dram_tensor("wkP", [128, 8, 3 * HC], BF16, kind="ExternalInput")
    wp = nc.dram_tensor("wp", [HC, C], BF16, kind="ExternalInput")
    nm = nc.dram_tensor("nm", [128, 128], BF16, kind="ExternalInput")
    ident = nc.dram_tensor("ident", [128, 128], BF16, kind="ExternalInput")
    po = nc.dram_tensor("po", [BT, C], F32, kind="ExternalOutput")
    if dbg:
        d_qt = nc.dram_tensor("d_qt", [B, 128, T], BF16, kind="ExternalOutput")
        d_kt = nc.dram_tensor("d_kt", [B, 128, T], BF16, kind="ExternalOutput")
        d_vt = nc.dram_tensor("d_vt", [B, 128, T], BF16, kind="ExternalOutput")
        d_ve = nc.dram_tensor(
            "d_ve", [B, 128, NH, NKB, D + 1], BF16, kind="ExternalOutput"
        )
        d_yt = nc.dram_tensor("d_yt", [B, HC, T], BF16, kind="ExternalOutput")
        d_acc = nc.dram_tensor(
            "d_acc", [B, NM, NH, D + 1, TQ], F32, kind="ExternalOutput"
        )
        d_rb = nc.dram_tensor(
            "d_rb", [B, NM, NH, D, TQ], F32, kind="ExternalOutput"
        )

    with TileContext(nc) as tc:
        with (
            tc.tile_pool(name="consts", bufs=1) as consts,
            tc.tile_pool(name="xin", bufs=4) as xin,
            tc.tile_pool(name="qk", bufs=2) as qkp,
            tc.tile_pool(name="vtp", bufs=1) as vtp,
            tc.tile_pool(name="vext", bufs=2) as vextp,
            tc.tile_pool(name="att", bufs=4) as attp,
            tc.tile_pool(name="yt", bufs=2) as ytp,
            tc.tile_pool(name="oout", bufs=4) as outp,
            tc.tile_pool(name="nrm", bufs=2) as nrmp,
            tc.tile_pool(name="big", bufs=3, space="PSUM") as bigp,
            tc.tile_pool(name="accp", bufs=2, space="PSUM") as accp,
        ):
            wq_sb = consts.tile([128, 8, 3 * HC], BF16)
            # critical first sliver: k-steps 0-1 of the qkv weights
            nc.sync.dma_start(wq_sb[:, 0:2], wkP.ap()[:, 0:2])
            wp_sb = consts.tile([HC, C], BF16)
            nm_sb = consts.tile([128, 128], BF16)
            id_sb = consts.tile([128, 128], BF16)
            ones_sb = consts.tile([128, 1], BF16)
            nc.gpsimd.memset(ones_sb[:], 1.0)

            def load_consts():
                # issued after the first x sliver so the PE-critical DMAs
                # reach the queues first
                nc.sync.dma_start(wq_sb[:, 2:8], wkP.ap()[:, 2:8])
                nc.sync.dma_start(id_sb[:], ident.ap())
                nc.sync.dma_start(nm_sb[:], nm.ap())
                nc.sync.dma_start(wp_sb[:], wp.ap())

            for b in range(B):
                # ---------------- QKV projection for batch b ----------------
                QT = qkp.tile([128, T], BF16, tag="QT")
                KT = qkp.tile([128, T], BF16, tag="KT")
                VT = vtp.tile([128, T], BF16, tag="VT")
                V_ext = vextp.tile([128, NH, NKB, D + 1], BF16, tag="vext")
                nc.vector.tensor_copy(
                    V_ext[:, :, :, D : D + 1],
                    ones_sb[:].to_broadcast((128, NH, NKB, 1)),
                )
                for tb in range(4):
                    xblk = xin.tile([128, 8, TQ], BF16, tag="xblk")
                    ci = b * 4 + tb
                    if ci == 0:
                        nc.sync.dma_start(xblk[:, 0:2], xP.ap()[:, ci, 0:2])
                        load_consts()
                        nc.sync.dma_start(xblk[:, 2:8], xP.ap()[:, ci, 2:8])
                    else:
                        nc.sync.dma_start(xblk[:], xP.ap()[:, ci])
                    ts = slice(tb * TQ, (tb + 1) * TQ)
                    # Q and K chunks accumulate into the two banks of tA
                    tA = bigp.tile([128, 2, TQ], F32, tag="big")
                    for j, dst in ((0, QT), (1, KT)):
                        for kt in range(8):
                            nc.tensor.matmul(
                                tA[:, j],
                                wq_sb[:, kt, j * 128 : (j + 1) * 128],
                                xblk[:, kt],
                                start=(kt == 0),
                                stop=(kt == 7),
                            )
                        nc.any.tensor_copy(dst[:, ts], tA[:, j])
                    # V chunk in bank 0 of tB; its transposes in bank 1
                    tB = bigp.tile([128, 2, TQ], F32, tag="big")
                    for kt in range(8):
                        nc.tensor.matmul(
                            tB[:, 0],
                            wq_sb[:, kt, 256:384],
                            xblk[:, kt],
                            start=(kt == 0),
                            stop=(kt == 7),
                        )
                    nc.any.tensor_copy(VT[:, ts], tB[:, 0])
                    # transposes write bf16 into the spare bank, bitcast view
                    tBv = tB[:, 1].bitcast(BF16)
                    for k4 in range(4):
                        kb = tb * 4 + k4
                        nc.tensor.transpose(
                            tBv[:, k4 * 128 : (k4 + 1) * 128],
                            VT[:, kb * 128 : (kb + 1) * 128],
                            id_sb[:],
                        )
                    nc.vector.tensor_copy(
                        V_ext[:, :, tb * 4 : (tb + 1) * 4, 0:D],
                        tBv[:, 0:512].rearrange("p (k h d) -> p h k d", k=4, h=NH),
                    )

                if dbg:
                    nc.sync.dma_start(d_qt.ap()[b], QT[:])
                    nc.sync.dma_start(d_kt.ap()[b], KT[:])
                    nc.sync.dma_start(d_vt.ap()[b], VT[:])
                    nc.sync.dma_start(d_ve.ap()[b], V_ext[:])

                # ---------------- attention + projection ----------------
                yT = ytp.tile([HC, T], BF16, tag="yT")
                for m in range(NM):
                    q0 = m * TQ
                    for h in range(NH):
                        hs = slice(h * D, (h + 1) * D)
                        acc = accp.tile([D + 1, TQ], F32, tag="acc")
                        ng = 2 * (m + 1)  # kb pairs
                        for g in range(ng):
                            diag = g >= ng - 2
                            qlo_s = 256 if g == ng - 1 else 0
                            pw = bigp.tile([128, 2, TQ], F32, tag="big")
                            for j2 in range(2):
                                kb = 2 * g + j2
                                nc.tensor.matmul(
                                    pw[:, j2, qlo_s:TQ],
                                    KT[hs, kb * 128 : (kb + 1) * 128],
                                    QT[hs, q0 + qlo_s : q0 + TQ],
                                    start=True,
                                    stop=not diag,
                                )
                            if diag:
                                # add -1e9 above the causal diagonal in PSUM
                                for j2 in range(2):
                                    jj = 2 * g + j2 - 4 * m
                                    ds = slice(jj * 128, (jj + 1) * 128)
                                    nc.tensor.matmul(
                                        pw[:, j2, ds],
                                        id_sb[:],
                                        nm_sb[:],
                                        start=False,
                                        stop=True,
                                    )
                            attT = attp.tile([128, 2, TQ], BF16, tag="attT")
                            nc.scalar.activation(
                                attT[:, :, qlo_s:TQ],
                                pw[:, :, qlo_s:TQ],
                                mybir.ActivationFunctionType.Exp,
                                scale=float(SCALE),
                            )
                            for j2 in range(2):
                                kb = 2 * g + j2
                                q_lo = max(kb - 4 * m, 0) * 128
                                nc.tensor.matmul(
                                    acc[:, q_lo:TQ],
                                    V_ext[:, h, kb, :],
                                    attT[:, j2, q_lo:TQ],
                                    start=(g == 0 and j2 == 0),
                                    stop=(g == ng - 1 and j2 == 1),
                                )
                        # yT[hs, q-slice] = numerator / denominator
                        if dbg:
                            da = nrmp.tile([D + 1, TQ], F32, tag="da")
                            nc.vector.tensor_copy(da[:], acc[:])
                            nc.sync.dma_start(d_acc.ap()[b, m, h], da[:])
                        # recip must read SBUF: custom-DVE ops misread PSUM on hw
                        d_sb = nrmp.tile([1, TQ], F32, tag="d")
                        nc.vector.tensor_copy(d_sb[:], acc[D : D + 1, :])
                        r = nrmp.tile([1, TQ], F32, tag="r")
                        nc.vector.reciprocal_approx_fast(r[:], d_sb[:])
                        rb = nrmp.tile([D, TQ], F32, tag="rb")
                        nc.gpsimd.partition_broadcast(rb[:], r[:])
                        if dbg:
                            nc.sync.dma_start(d_rb.ap()[b, m, h], rb[:])
                        nc.vector.tensor_tensor(
                            yT[hs, q0 : q0 + TQ],
                            acc[0:D, :],
                            rb[:],
                            mybir.AluOpType.mult,
                        )
                    # out-projection for this query tile
                    for t4 in range(4):
                        t0 = q0 + t4 * 128
                        pj = bigp.tile([128, 2, TQ], F32, tag="big")
                        for nn in range(2):
                            nc.tensor.matmul(
                                pj[:, nn],
                                yT[:, t0 : t0 + 128],
                                wp_sb[:, nn * TQ : (nn + 1) * TQ],
                                start=True,
                                stop=True,
                            )
                        ot = outp.tile([128, C], F32, tag="ot")
                        nc.any.tensor_copy(
                            ot[:], pj[:].rearrange("p a b -> p (a b)")
                        )
                        nc.sync.dma_start(
                            po.ap()[b * T + t0 : b * T + t0 + 128, :], ot[:]
                        )
                if dbg:
                    nc.sync.dma_start(d_yt.ap()[b], yT[:])

    nc.compile()
    return nc


def prepare_in_maps(x, w_qkv, w_proj):
    x = np.ascontiguousarray(x, dtype=np.float32)
    w_qkv = np.ascontiguousarray(w_qkv, dtype=np.float32)
    w_proj = np.ascontiguousarray(w_proj, dtype=np.float32)

    xT = x.reshape(BT, C).T  # [C, BT]
    # [o, p, chunk, t] -> [p, chunk, o, t]
    xPh = np.ascontiguousarray(
        xT.reshape(8, 128, NC, TQ).transpose(1, 2, 0, 3)
    ).astype(BF)
    nmh = np.where(
        np.arange(128)[:, None] > np.arange(128)[None, :], -1e9, 0.0
    ).astype(BF)
    identh = np.eye(128, dtype=np.float32).astype(BF)

    in_maps = []
    for i in range(N_CORES):
        cs = slice(HC * i, HC * (i + 1))
        wk_i = np.concatenate(
            [w_qkv[:, cs], w_qkv[:, C:][:, cs], w_qkv[:, 2 * C :][:, cs]], axis=1
        )
        wkPh = np.ascontiguousarray(
            wk_i.reshape(8, 128, 3 * HC).transpose(1, 0, 2)
        ).astype(BF)
        wp_i = np.ascontiguousarray(w_proj[cs, :]).astype(BF)
        in_maps.append(
            {"xP": xPh, "wkP": wkPh, "wp": wp_i, "nm": nmh, "ident": identh}
        )
    return in_maps


_CACHED_NC = None


def kernel(x: np.ndarray, w_qkv: np.ndarray, w_proj: np.ndarray) -> np.ndarray:
    global _CACHED_NC
    if _CACHED_NC is None:
        _CACHED_NC = build_program()
    nc = _CACHED_NC

    in_maps = prepare_in_maps(x, w_qkv, w_proj)
    res = run_bass_kernel_spmd(nc, in_maps, core_ids=list(range(N_CORES)))
    total = np.zeros((BT, C), dtype=np.float64)
    for i in range(N_CORES):
        total += res.results[i]["po"]
    return total.astype(np.float32).reshape(B, T, C)


if __name__ == "__main__":
    rng = np.random.default_rng(0)
    x = rng.standard_normal((B, T, C), dtype=np.float32)
    w_qkv = rng.standard_normal((C, 3 * C), dtype=np.float32) / np.sqrt(C)
    w_proj = rng.standard_normal((C, C), dtype=np.float32) / np.sqrt(C)
    out = kernel(x=x, w_qkv=w_qkv, w_proj=w_proj)
    print(out.shape, out.dtype, np.abs(out).mean())


# revision 11
# speedup vs baseline: 1.1986x; 1.1986x over previous
"""Causal multi-head attention (B=2, T=2048, C=1024, H=16, D=64) on 8 TRN2 cores.

Sharding: 2 heads per core (head-parallel). Each core computes its heads'
QKV projection, block-causal attention, and a partial out-projection; the
host sums the 8 partials.

v3: single software pipeline over the 8 (batch, m) query tiles:
    qkv(chunk 0); for i in 0..7: attn(i), qkv(i+1), proj(i-1); proj(7)
  so the ACT-bound attention inner loop always has independent QKV/proj
  matmuls adjacent in priority to fill PE gaps, and the projection runs one
  tile behind the normalize chain instead of stalling the in-order PE queue.
  All PSUM->SBUF copies are forced onto DVE (ACT does exp only).
  bf16 operands everywhere (PSUM fp32); causal column restriction on the
  2nd diagonal pair (scores + exp); triangular mask folded into PSUM via
  identity-stationary matmuls; V transposes packed into the spare PSUM bank
  of the V-chunk tile; partial outputs shipped bf16.
"""
import sys

sys.path.insert(0, "/opt/trn_rl_repo")

import numpy as np
import ml_dtypes

import concourse.bass as bass
import concourse.mybir as mybir
from concourse import bacc
from concourse.tile import TileContext
from concourse.bass_utils import run_bass_kernel_spmd

N_CORES = 8
B, T, C = 2, 2048, 1024
D = 64          # head dim
NH = 2          # heads per core
HC = NH * D     # 128: head-channels per core
BT = B * T      # 4096
TQ = 512        # query tile
NM = T // TQ    # 4 query tiles per batch
NKB = T // 128  # 16 key blocks per batch
NC = B * NM     # 8 x-chunks of 512 tokens
F32 = mybir.dt.float32
BF16 = mybir.dt.bfloat16
SCALE = 1.0 / np.sqrt(D)  # 0.125
BF = ml_dtypes.bfloat16


def build_program(dbg: bool = False):
    nc = bacc.Bacc("TRN2", target_bir_lowering=False, debug=False)

    # xP[p, chunk, o, t] = x.T[o*128+p, chunk*512+t] -- per-partition contiguous
    xP = nc.dram_tensor("xP", [128, NC, 8, TQ], BF16, kind="ExternalInput")
    # wkP[p, o, m] = wk[o*128+p, m], m = [Q(128) | K(128) | V(128)] head cols
    wkP = nc.dram_tensor("wkP", [128, 8, 3 * HC], BF16, kind="ExternalInput")
    wp = nc.dram_tensor("wp", [HC, C], BF16, kind="ExternalInput")
    nm = nc.dram_tensor("nm", [128, 128], BF16, kind="ExternalInput")
    ident = nc.dram_tensor("ident", [128, 128], BF16, kind="ExternalInput")
    po = nc.dram_tensor("po", [BT, C], BF16, kind="ExternalOutput")
    if dbg:
        d_yt = nc.dram_tensor("d_yt", [B, HC, T], BF16, kind="ExternalOutput")
        d_acc = nc.dram_tensor(
            "d_acc", [B, NM, NH, D + 1, TQ], F32, kind="ExternalOutput"
        )

    with TileContext(nc) as tc:
        with (
            tc.tile_pool(name="consts", bufs=1) as consts,
            tc.tile_pool(name="xin", bufs=4) as xin,
            tc.tile_pool(name="qk", bufs=2) as qkp,
            tc.tile_pool(name="vtp", bufs=2) as vtp,
            tc.tile_pool(name="vext", bufs=2) as vextp,
            tc.tile_pool(name="att", bufs=4) as attp,
            tc.tile_pool(name="yt", bufs=2) as ytp,
            tc.tile_pool(name="oout", bufs=4) as outp,
            tc.tile_pool(name="nrm", bufs=2) as nrmp,
            tc.tile_pool(name="big", bufs=3, space="PSUM") as bigp,
            tc.tile_pool(name="accp", bufs=2, space="PSUM") as accp,
        ):
            wq_sb = consts.tile([128, 8, 3 * HC], BF16)
            # critical first sliver: k-steps 0-1 of the qkv weights
            nc.sync.dma_start(wq_sb[:, 0:2], wkP.ap()[:, 0:2])
            wp_sb = consts.tile([HC, C], BF16)
            nm_sb = consts.tile([128, 128], BF16)
            id_sb = consts.tile([128, 128], BF16)
            ones_sb = consts.tile([128, 1], BF16)
            nc.gpsimd.memset(ones_sb[:], 1.0)

            def load_consts():
                # issued after the first x sliver so the PE-critical DMAs
                # reach the queues first
                nc.sync.dma_start(wq_sb[:, 2:8], wkP.ap()[:, 2:8])
                nc.sync.dma_start(id_sb[:], ident.ap())
                nc.sync.dma_start(nm_sb[:], nm.ap())
                nc.sync.dma_start(wp_sb[:], wp.ap())

            bt = [dict(), dict()]  # per-batch live tiles

            def emit_qkv_chunk(ci):
                b, tb = divmod(ci, 4)
                s = bt[b]
                if tb == 0:
                    s["QT"] = qkp.tile([128, T], BF16, tag="QT", name="QT")
                    s["KT"] = qkp.tile([128, T], BF16, tag="KT", name="KT")
                    s["VT"] = vtp.tile([128, T], BF16, tag="VT", name="VT")
                    s["V"] = vextp.tile([128, NH, NKB, D + 1], BF16, tag="ve", name="Vx")
                    nc.vector.tensor_copy(
                        s["V"][:, :, :, D : D + 1],
                        ones_sb[:].to_broadcast((128, NH, NKB, 1)),
                    )
                xblk = xin.tile([128, 8, TQ], BF16, tag="xblk")
                if ci == 0:
                    nc.sync.dma_start(xblk[:, 0:2], xP.ap()[:, ci, 0:2])
                    load_consts()
                    nc.sync.dma_start(xblk[:, 2:8], xP.ap()[:, ci, 2:8])
                else:
                    nc.sync.dma_start(xblk[:], xP.ap()[:, ci])
                ts = slice(tb * TQ, (tb + 1) * TQ)
                # Q and K chunks accumulate into the two banks of tA
                tA = bigp.tile([128, 2, TQ], F32, tag="big")
                for j, dst in ((0, s["QT"]), (1, s["KT"])):
                    for kt in range(8):
                        nc.tensor.matmul(
                            tA[:, j],
                            wq_sb[:, kt, j * 128 : (j + 1) * 128],
                            xblk[:, kt],
                            start=(kt == 0),
                            stop=(kt == 7),
                        )
                    nc.vector.tensor_copy(dst[:, ts], tA[:, j])
                # V chunk in bank 0 of tB; its transposes (bf16) in bank 1
                tB = bigp.tile([128, 2, TQ], F32, tag="big")
                for kt in range(8):
                    nc.tensor.matmul(
                        tB[:, 0],
                        wq_sb[:, kt, 256:384],
                        xblk[:, kt],
                        start=(kt == 0),
                        stop=(kt == 7),
                    )
                nc.vector.tensor_copy(s["VT"][:, ts], tB[:, 0])
                tBv = tB[:, 1].bitcast(BF16)
                for k4 in range(4):
                    kb = tb * 4 + k4
                    nc.tensor.transpose(
                        tBv[:, k4 * 128 : (k4 + 1) * 128],
                        s["VT"][:, kb * 128 : (kb + 1) * 128],
                        id_sb[:],
                    )
                nc.vector.tensor_copy(
                    s["V"][:, :, tb * 4 : (tb + 1) * 4, 0:D],
                    tBv[:, 0:512].rearrange("p (k h d) -> p h k d", k=4, h=NH),
                )

            def emit_attn(i):
                b, m = divmod(i, 4)
                s = bt[b]
                if m == 0:
                    s["yT"] = ytp.tile([HC, T], BF16, tag="yT", name="yT")
                QT, KT, V_ext, yT = s["QT"], s["KT"], s["V"], s["yT"]
                q0 = m * TQ
                for h in range(NH):
                    hs = slice(h * D, (h + 1) * D)
                    acc = accp.tile([D + 1, TQ], F32, tag="acc")
                    ng = 2 * (m + 1)  # kb pairs
                    for g in range(ng):
                        diag = g >= ng - 2
                        qlo_s = 256 if g == ng - 1 else 0
                        pw = bigp.tile([128, 2, TQ], F32, tag="big")
                        for j2 in range(2):
                            kb = 2 * g + j2
                            nc.tensor.matmul(
                                pw[:, j2, qlo_s:TQ],
                                KT[hs, kb * 128 : (kb + 1) * 128],
                                QT[hs, q0 + qlo_s : q0 + TQ],
                                start=True,
                                stop=not diag,
                            )
                        if diag:
                            # add -1e9 above the causal diagonal in PSUM
                            for j2 in range(2):
                                jj = 2 * g + j2 - 4 * m
                                ds = slice(jj * 128, (jj + 1) * 128)
                                nc.tensor.matmul(
                                    pw[:, j2, ds],
                                    id_sb[:],
                                    nm_sb[:],
                                    start=False,
                                    stop=True,
                                )
                        attT = attp.tile([128, 2, TQ], BF16, tag="attT")
                        nc.scalar.activation(
                            attT[:, :, qlo_s:TQ],
                            pw[:, :, qlo_s:TQ],
                            mybir.ActivationFunctionType.Exp,
                            scale=float(SCALE),
                        )
                        for j2 in range(2):
                            kb = 2 * g + j2
                            q_lo = max(kb - 4 * m, 0) * 128
                            nc.tensor.matmul(
                                acc[:, q_lo:TQ],
                                V_ext[:, h, kb, :],
                                attT[:, j2, q_lo:TQ],
                                start=(g == 0 and j2 == 0),
                                stop=(g == ng - 1 and j2 == 1),
                            )
                    # yT[hs, q-slice] = numerator / denominator
                    if dbg:
                        da = nrmp.tile([D + 1, TQ], F32, tag="da")
                        nc.vector.tensor_copy(da[:], acc[:])
                        nc.sync.dma_start(d_acc.ap()[b, m, h], da[:])
                    # recip must read SBUF: custom-DVE ops misread PSUM on hw
                    d_sb = nrmp.tile([1, TQ], F32, tag="d")
                    nc.vector.tensor_copy(d_sb[:], acc[D : D + 1, :])
                    r = nrmp.tile([1, TQ], F32, tag="r")
                    nc.vector.reciprocal_approx_fast(r[:], d_sb[:])
                    rb = nrmp.tile([D, TQ], F32, tag="rb")
                    nc.gpsimd.partition_broadcast(rb[:], r[:])
                    nc.vector.tensor_tensor(
                        yT[hs, q0 : q0 + TQ],
                        acc[0:D, :],
                        rb[:],
                        mybir.AluOpType.mult,
                    )
                if dbg and m == NM - 1:
                    nc.sync.dma_start(d_yt.ap()[b], yT[:])

            def emit_proj(i):
                b, m = divmod(i, 4)
                yT = bt[b]["yT"]
                for t4 in range(4):
                    t0 = m * TQ + t4 * 128
                    pj = bigp.tile([128, 2, TQ], F32, tag="big")
                    for nn in range(2):
                        nc.tensor.matmul(
                            pj[:, nn],
                            yT[:, t0 : t0 + 128],
                            wp_sb[:, nn * TQ : (nn + 1) * TQ],
                            start=True,
                            stop=True,
                        )
                    ot = outp.tile([128, C], BF16, tag="ot")
                    nc.vector.tensor_copy(
                        ot[:], pj[:].rearrange("p a b -> p (a b)")
                    )
                    nc.sync.dma_start(
                        po.ap()[b * T + t0 : b * T + t0 + 128, :], ot[:]
                    )

            emit_qkv_chunk(0)
            for i in range(B * NM):
                emit_attn(i)
                if i + 1 < B * NM:
                    emit_qkv_chunk(i + 1)
                if i >= 1:
                    emit_proj(i - 1)
            emit_proj(B * NM - 1)

    nc.compile()
    return nc


def prepare_in_maps(x, w_qkv, w_proj):
    x = np.ascontiguousarray(x, dtype=np.float32)
    w_qkv = np.ascontiguousarray(w_qkv, dtype=np.float32)
    w_proj = np.ascontiguousarray(w_proj, dtype=np.float32)

    xT = x.reshape(BT, C).T  # [C, BT]
    # [o, p, chunk, t] -> [p, chunk, o, t]
    xPh = np.ascontiguousarray(
        xT.reshape(8, 128, NC, TQ).transpose(1, 2, 0, 3)
    ).astype(BF)
    nmh = np.where(
        np.arange(128)[:, None] > np.arange(128)[None, :], -1e9, 0.0
    ).astype(BF)
    identh = np.eye(128, dtype=np.float32).astype(BF)

    in_maps = []
    for i in range(N_CORES):
        cs = slice(HC * i, HC * (i + 1))
        wk_i = np.concatenate(
            [w_qkv[:, cs], w_qkv[:, C:][:, cs], w_qkv[:, 2 * C :][:, cs]], axis=1
        )
        wkPh = np.ascontiguousarray(
            wk_i.reshape(8, 128, 3 * HC).transpose(1, 0, 2)
        ).astype(BF)
        wp_i = np.ascontiguousarray(w_proj[cs, :]).astype(BF)
        in_maps.append(
            {"xP": xPh, "wkP": wkPh, "wp": wp_i, "nm": nmh, "ident": identh}
        )
    return in_maps


_CACHED_NC = None


def kernel(x: np.ndarray, w_qkv: np.ndarray, w_proj: np.ndarray) -> np.ndarray:
    global _CACHED_NC
    if _CACHED_NC is None:
        _CACHED_NC = build_program()
    nc = _CACHED_NC

    in_maps = prepare_in_maps(x, w_qkv, w_proj)
    res = run_bass_kernel_spmd(nc, in_maps, core_ids=list(range(N_CORES)))
    total = np.zeros((BT, C), dtype=np.float64)
    for i in range(N_CORES):
        total += np.asarray(res.results[i]["po"], dtype=np.float32)
    return total.astype(np.float32).reshape(B, T, C)


if __name__ == "__main__":
    rng = np.random.default_rng(0)
    x = rng.standard_normal((B, T, C), dtype=np.float32)
    w_qkv = rng.standard_normal((C, 3 * C), dtype=np.float32) / np.sqrt(C)
    w_proj = rng.standard_normal((C, C), dtype=np.float32) / np.sqrt(C)
    out = kernel(x=x, w_qkv=w_qkv, w_proj=w_proj)
    print(out.shape, out.dtype, np.abs(out).mean())
